# revision 2
# baseline (speedup 1.0000x reference)
"""LMU (Legendre Memory Unit) Trainium2 Bass kernel.

Full-input contract: kernel(**inputs) takes the unsharded inputs from
setup_inputs() and returns the full (64, 2048, 512) output. Data-parallel:
batch 64 -> 8 cores x 8; the scan z' = [tanh|id](pc_t + z@W) uses a host-
folded W (768x768) and on-device pc = x@Wx, 2048 sequential steps.

v2's contiguous DMA layouts, plus the scan runs in bf16 with compensated
precision (fp32 weights cost ~550ns/matmul from the 2-pass self-load; bf16
with FWL costs ~65ns):

  W = Whi + Wlo (both bf16, host-split)     -> z@W as two bf16 passes
  h state: bf16 (tanh output; error ~7e-3 at T=2048, inside 2e-2 budget)
  m state: DUAL bf16 (mhi + mlo) - single-bf16 m integrates rounding noise
           through the marginally-stable Legendre dynamics (0.1 rel-err);
           the mlo@Whi correction pass (12 MMs) fixes it (7e-3 total).

84 bf16 matmuls/step vs v1's 36 fp32: ~5.5us/step vs ~20us.
PSUM accumulation and the pc path stay fp32.
"""

import os
import numpy as np
from contextlib import ExitStack

import concourse.bass as bass
import concourse.bacc as bacc
import concourse.tile as tile
import concourse.mybir as mybir
from concourse.bass_utils import run_bass_kernel_spmd

F32 = mybir.dt.float32
BF16 = mybir.dt.bfloat16

B = 8          # batch per core
NCORES = 8
D = 256        # input dim
H = 512        # hidden units
MO = 256       # memory order
Z = H + MO     # 768 stacked state
KT = Z // 128  # 6 K-tiles
MT = Z // 128  # 6 M-tiles
HT = H // 128  # 4 h tiles
CT = 128       # steps per chunk

_cache = {}


def _build(T: int, repeat: int = 1):
    assert T % (2 * CT) == 0
    nc = bacc.Bacc("TRN2", target_bir_lowering=False, debug=False)

    x_d = nc.dram_tensor("x", [2, 128, B, T], F32, kind="ExternalInput")
    whi_d = nc.dram_tensor("Whi", [Z, Z], BF16, kind="ExternalInput")
    wlo_d = nc.dram_tensor("Wlo", [Z, Z], BF16, kind="ExternalInput")
    wx_d = nc.dram_tensor("Wx", [D, Z], F32, kind="ExternalInput")
    out_d = nc.dram_tensor("out", [HT, 128, B, T], BF16, kind="ExternalOutput")

    with tile.TileContext(nc) as tc, ExitStack() as ctx:
        const = ctx.enter_context(tc.tile_pool(name="const", bufs=1))
        whi_sb = const.tile([128, KT * Z], BF16)  # tile (kt,mt) at (kt*MT+mt)*128
        wlo_sb = const.tile([128, KT * Z], BF16)
        wx_sb = const.tile([128, 2 * Z], F32)
        # x chunk: [128, (k2, b, t)]
        xbuf = [const.tile([128, 2 * B * CT], F32, name=f"x{i}", tag=f"x{i}") for i in range(2)]
        # pc chunk: [128, (mt, b, t)]
        pcbuf = [const.tile([128, MT * B * CT], F32, name=f"pc{i}", tag=f"pc{i}") for i in range(2)]
        # h chunk: [128, (mt, b, t)] bf16
        hbuf = [const.tile([128, HT * B * CT], BF16, name=f"h{i}", tag=f"h{i}") for i in range(2)]
        mhib = [const.tile([128, 2 * B], BF16, name=f"mhi{i}", tag=f"mhi{i}") for i in range(2)]
        mlob = [const.tile([128, 2 * B], BF16, name=f"mlo{i}", tag=f"mlo{i}") for i in range(2)]

        pspool = ctx.enter_context(tc.tile_pool(name="ps", bufs=1, space="PSUM"))
        ps_scan = pspool.tile([128, MT * 512], F32)   # 6 banks, one per M-tile
        pcps = ctx.enter_context(tc.tile_pool(name="pcps", bufs=2, space="PSUM"))
        tmp_pool = ctx.enter_context(tc.tile_pool(name="tmp", bufs=4))

        # --- prologue: weights + state init ---
        for kt in range(KT):
            nc.sync.dma_start(
                whi_sb[:, kt * Z:(kt + 1) * Z], whi_d.ap()[kt * 128:(kt + 1) * 128, :])
            nc.sync.dma_start(
                wlo_sb[:, kt * Z:(kt + 1) * Z], wlo_d.ap()[kt * 128:(kt + 1) * 128, :])
        for k2 in range(2):
            nc.sync.dma_start(
                wx_sb[:, k2 * Z:(k2 + 1) * Z], wx_d.ap()[k2 * 128:(k2 + 1) * 128, :])
        nc.vector.memset(mhib[1][:], 0.0)                        # m_{-1} = 0
        nc.vector.memset(mlob[1][:], 0.0)
        # h_{-1} = 0: the t=CT-1 column of hbuf[1], strided view
        hv1 = hbuf[1][:].rearrange("p (m b t) -> p m b t", m=HT, b=B, t=CT)
        nc.vector.memset(hv1[:, :, :, CT - 1], 0.0)

        def dma_x(xb, toff):
            dstv = xb[:].rearrange("p (k b t) -> p k b t", k=2, b=B, t=CT)
            for k2 in range(2):
                nc.sync.dma_start(
                    dstv[:, k2], x_d.ap()[k2, :, :, bass.ds(toff, CT)])

        def pc_gemm(xb, pcb):
            xv = xb[:].rearrange("p (k b t) -> p k b t", k=2, b=B, t=CT)
            pcv = pcb[:].rearrange("p (m b t) -> p m b t", m=MT, b=B, t=CT)
            for mt in range(MT):
                for ns in range(2):
                    ps = pcps.tile([128, 512], F32, name="pcp", tag="pcps")
                    psv = ps[:].rearrange("p (b t) -> p b t", b=B, t=64)
                    for k2 in range(2):
                        nc.tensor.matmul(
                            ps[:],
                            wx_sb[:, k2 * Z + mt * 128: k2 * Z + (mt + 1) * 128],
                            xv[:, k2, :, ns * 64:(ns + 1) * 64],
                            start=(k2 == 0), stop=(k2 == 1))
                    nc.vector.tensor_copy(
                        pcv[:, mt, :, ns * 64:(ns + 1) * 64], psv[:])
            return

        def scan_chunk(hb, hb_prev, pcb):
            psv = ps_scan[:].rearrange("p (m x) -> p m x", m=MT, x=512)
            pcv = pcb[:].rearrange("p (m b t) -> p m b t", m=MT, b=B, t=CT)
            hv = hb[:].rearrange("p (m b t) -> p m b t", m=HT, b=B, t=CT)
            hv_prev = hb_prev[:].rearrange("p (m b t) -> p m b t", m=HT, b=B, t=CT)
            for t in range(CT):
                hprev = (hv_prev if t == 0 else hv)[:, :, :, (t - 1) % CT]
                mhi_in = mhib[1 - (t % 2)]
                mlo_in = mlob[1 - (t % 2)]
                mhi_out = mhib[t % 2]
                mlo_out = mlob[t % 2]
                for mt in range(MT):
                    # accumulation group for bank mt, m-rows first (ready
                    # earlier than h, which goes through tanh on ACT):
                    # (rhs, weight) pairs in issue order
                    prog = []
                    for kt in (HT, HT + 1):
                        prog.append((mhi_in[:, (kt - HT) * B:(kt - HT + 1) * B], whi_sb, kt))
                        prog.append((mhi_in[:, (kt - HT) * B:(kt - HT + 1) * B], wlo_sb, kt))
                        prog.append((mlo_in[:, (kt - HT) * B:(kt - HT + 1) * B], whi_sb, kt))
                    for kt in range(HT):
                        prog.append((hprev[:, kt, :], whi_sb, kt))
                        prog.append((hprev[:, kt, :], wlo_sb, kt))
                    for i, (rhs, wsb, kt) in enumerate(prog):
                        w_tile = wsb[:, (kt * MT + mt) * 128:(kt * MT + mt + 1) * 128]
                        nc.tensor.matmul(
                            ps_scan[:, mt * 512: mt * 512 + B], w_tile, rhs,
                            start=(i == 0), stop=(i == len(prog) - 1))
                # h' = tanh(psum_h + pc_h)
                tmp = tmp_pool.tile([128, 4 * B], F32, name="tmph", tag="tmph")
                nc.vector.tensor_add(
                    tmp[:].rearrange("p (m b) -> p m b", m=HT, b=B),
                    psv[:, 0:HT, 0:B], pcv[:, 0:HT, :, t])
                nc.scalar.activation(
                    hv[:, :, :, t], tmp[:].rearrange("p (m b) -> p m b", m=HT, b=B),
                    mybir.ActivationFunctionType.Tanh)
                # m' = psum_m + pc_m (fp32), then dual-store bf16 hi+lo
                tmpm = tmp_pool.tile([128, 2 * B], F32, name="tmpm", tag="tmpm")
                nc.vector.tensor_add(
                    tmpm[:].rearrange("p (m b) -> p m b", m=2, b=B),
                    psv[:, HT:MT, 0:B], pcv[:, HT:MT, :, t])
                nc.vector.tensor_copy(mhi_out[:], tmpm[:])       # rounds to bf16
                nc.vector.tensor_sub(mlo_out[:], tmpm[:], mhi_out[:])

        def dma_out(hb, toff):
            hv = hb[:].rearrange("p (m b t) -> p m b t", m=HT, b=B, t=CT)
            for mt in range(HT):
                nc.sync.dma_start(
                    out_d.ap()[mt, :, :, bass.ds(toff, CT)], hv[:, mt])

        def body(toff):
            dma_x(xbuf[0], toff)
            pc_gemm(xbuf[0], pcbuf[0])
            dma_x(xbuf[1], toff + CT)
            scan_chunk(hbuf[0], hbuf[1], pcbuf[0])
            dma_out(hbuf[0], toff)
            pc_gemm(xbuf[1], pcbuf[1])
            scan_chunk(hbuf[1], hbuf[0], pcbuf[1])
            dma_out(hbuf[1], toff + CT)

        if repeat > 1:
            with tc.For_i(0, repeat) as _r:
                with tc.For_i(0, T, 2 * CT) as toff:
                    body(toff)
        else:
            with tc.For_i(0, T, 2 * CT) as toff:
                body(toff)

    nc.compile()
    return nc


def _host_weights(inputs):
    ie = np.asarray(inputs["input_encoders"], np.float64)
    he = np.asarray(inputs["hidden_encoders"], np.float64)
    me = np.asarray(inputs["memory_encoders"], np.float64)
    ik = np.asarray(inputs["input_kernel"], np.float64)
    hk = np.asarray(inputs["hidden_kernel"], np.float64)
    mk = np.asarray(inputs["memory_kernel"], np.float64)
    AT = np.asarray(inputs["AT"], np.float64)
    BT = np.asarray(inputs["BT"], np.float64)
    ATI = AT + np.eye(MO)
    mk2 = ATI @ mk
    g = BT @ mk
    W = np.zeros((Z, Z))
    W[0:H, 0:H] = hk + he @ g
    W[H:Z, 0:H] = mk2 + me @ g
    W[0:H, H:Z] = he @ BT
    W[H:Z, H:Z] = ATI + me @ BT
    Wx = np.zeros((D, Z))
    Wx[:, 0:H] = ik + ie @ g
    Wx[:, H:Z] = ie @ BT
    return W.astype(np.float32), Wx.astype(np.float32)


def kernel(**inputs):
    import ml_dtypes
    x = np.ascontiguousarray(np.asarray(inputs["x"], np.float32))
    Bfull, T, _ = x.shape
    W, Wx = _host_weights(inputs)
    Whi = W.astype(ml_dtypes.bfloat16)
    Wlo = (W - Whi.astype(np.float32)).astype(ml_dtypes.bfloat16)

    rep = int(os.environ.get("LMU_REPEAT", "1"))
    key = (T, rep)
    if key not in _cache:
        _cache[key] = _build(T, repeat=rep)
    nc = _cache[key]

    per = Bfull // NCORES
    # host pre-transpose: [core][k2, d, b, t] <- x[core*per+b, t, k2*128+d]
    xr = x.reshape(NCORES, per, T, 2, 128).transpose(0, 3, 4, 1, 2)
    in_maps = [
        {"x": np.ascontiguousarray(xr[c]), "Whi": Whi, "Wlo": Wlo, "Wx": Wx}
        for c in range(NCORES)
    ]
    res = run_bass_kernel_spmd(nc, in_maps, core_ids=list(range(NCORES)))
    # out_dev [mt, d, b, t] -> [b, t, mt*128+d]
    outs = [np.asarray(r["out"]).astype(np.float32).transpose(2, 3, 0, 1).reshape(per, T, H)
            for r in res.results]
    return np.concatenate(outs, axis=0)
